# revision 5
# baseline (speedup 1.0000x reference)
"""Cubic B-spline FFD upsampling v2 (stride 5, 42^3 control grid -> 192^3).

Full input v: (4, 3, 42, 42, 42) f32 -> output (4, 3, 192, 192, 192) f32.
Sharding: 8 cores = batch(4) x first-spatial-axis halves(2); each core emits
a (3, 96, 192, 192) fp16 slab (tolerance 2e-2; fp16 pipeline err ~2e-3).

v2 changes vs v1 (copy engines ACT+DVE are the bottleneck; only they can
read PSUM, combined ~2.16 elem/ns/partition):
  s1 ch-packing: matmul M = 126 = (3ch x 42z), one matmul per x value ->
      psum [126, 192] and ONE copy per tile instead of 2 copies x 3ch
      (stage-1 copy work 13.8k -> 4.6k per-partition elems).
  s3 512-streams: rhs = slab 512-windows (ignoring oyq boundaries), psum
      tiles [128, 1536] = 3 matmuls, copy 1536 contiguous -> 54 big copies
      instead of 108 strided 768-elem ones.
  out-DMAs issue from SP only, deferred one group (a DMACopy's sem-waits
      hold the issuing SEQ, which would stall ACT's copy stream).
  s2 runs on 1-bank double-buffered psum tiles (2 matmuls each) so its
      fill->copy chain pipelines instead of serializing on one buffer.
  J double-buffered so s1 of rep r+1 overlaps the s3 tail of rep r;
  ob pool 6-deep so copies ride through DMA jitter under HBM contention.
"""

import numpy as np

import concourse.bass as bass
import concourse.mybir as mybir
import concourse.tile as tile
from concourse import bacc
from concourse.bass_utils import run_bass_kernel_spmd

F32 = mybir.dt.float32
F16 = mybir.dt.float16

N_CORES = 8

_NC_CACHE = None


def _bspline_B() -> np.ndarray:
    """B[o, c]: weight of control point c on cropped output sample o."""
    n = 19
    t = np.abs((np.arange(n) - 9) / 5.0)
    w = np.where(
        t < 1.0,
        2.0 / 3.0 + (0.5 * t - 1.0) * t**2,
        np.where(t < 2.0, -((t - 2.0) ** 3) / 6.0, 0.0),
    )
    o = np.arange(192)[:, None]
    c = np.arange(42)[None, :]
    k = 5 * c + 4 - o
    B = np.where((k >= 0) & (k < 19), w[np.clip(k, 0, 18)], 0.0)
    return np.ascontiguousarray(B, dtype=np.float32)


def _build_nc(reps: int = 1):
    global _NC_CACHE
    if reps == 1 and _NC_CACHE is not None:
        return _NC_CACHE

    nc = bacc.Bacc("TRN2", target_bir_lowering=False, debug=False, num_devices=N_CORES)
    inp = nc.dram_tensor("inp", [42, 4608], F16, kind="ExternalInput").ap()
    byT = nc.dram_tensor("byT", [128, 192], F16, kind="ExternalInput").ap()
    bx3 = nc.dram_tensor("bx3", [96, 384], F16, kind="ExternalInput").ap()
    out = nc.dram_tensor("out", [3, 96, 192, 192], F16, kind="ExternalOutput").ap()

    # out[ch, 32b+oxr, 48Q+16wpr+oo, z] viewed as (c wpr Q | oxr | b (oo z));
    # (oo z) = 16 consecutive oy rows x 192 oz = one contiguous 6KB run
    # (wp-pairs share one ob tile -> 36 DMAs x 48 descriptors per rep).
    outv = out.rearrange(
        "c (b oxr) (Q wpr oo) z -> c wpr Q oxr b (oo z)",
        b=3, oxr=32, Q=4, wpr=3, oo=16,
    )

    copy_load = [0.0, 0.0]  # accumulated cost on [DVE, ACT]

    def copy(dst, src):
        n = src.free_size()
        cost_dve = (n + 120.0) / 0.96
        cost_act = (n + 222.0) / 1.2
        if copy_load[0] + cost_dve <= copy_load[1] + cost_act:
            copy_load[0] += cost_dve
            nc.vector.tensor_copy(dst, src)
        else:
            copy_load[1] += cost_act
            nc.scalar.copy(dst, src)

    dma_k = [0]
    dma_pending = []

    def out_dma(dst, src):
        # deferred issue: queue now, fire when the producing copies are long
        # done -- a DMACopy's sem-waits hold its issuing SEQ, so issuing
        # right after the copies would stall that engine's copy stream.
        dma_pending.append((dst, src))

    def flush_dmas():
        for dst, src in dma_pending:
            dma_k[0] += 1
            nc.sync.dma_start(dst, src)
        dma_pending.clear()

    with tile.TileContext(nc) as tc:
        with (
            tc.tile_pool(name="const", bufs=1) as cpool,
            tc.tile_pool(name="jp", bufs=2) as jp,
            tc.tile_pool(name="slabs", bufs=6) as slabs,
            tc.tile_pool(name="obp", bufs=6) as obp,
            tc.tile_pool(name="ps3", bufs=2, space="PSUM") as ps3p,
            tc.tile_pool(name="ps2", bufs=2, space="PSUM") as ps2p,
        ):
            tB = cpool.tile([128, 192], F16, name="tB")
            tBx = cpool.tile([96, 384], F16, name="tBx")
            tI = cpool.tile([42, 4608], F16, name="tI")
            nc.sync.dma_start(tB[:], byT[:])
            nc.sync.dma_start(tBx[:], bx3[:])
            nc.sync.dma_start(tI[:], inp[:])

            for _rep in range(reps):
                # stage 1: expand y. Matmul base partitions must be 0/32/64,
                # so ch0+ch1 share J_A (partition bands 0-41 / 64-105) with
                # M = 128 = (2ch x 64 slots) per x -> ONE full-128 copy per
                # psum tile; ch2 goes baseline-style (M = (2x x 64z)) to J_B
                # with 2 band-copies per tile. tB holds B^T at rows 0-41 AND
                # 64-105 so lhsT/rhs base partitions can match per ch-band.
                J_A = jp.tile([128, 4608], F16, name="J_A")
                J_B = jp.tile([128, 4608], F16, name="J_B")
                JAw = J_A.rearrange("p (oyq Q x) -> p x Q oyq", oyq=48, Q=4, x=24)
                JBv = J_B.rearrange(
                    "p (oyq Q xp e) -> p e xp Q oyq", oyq=48, Q=4, xp=12, e=2
                )
                # ch0+ch1: interleave x-triples through ps3 and x-pairs
                # through ps2 so the s1 burst pipelines 4 tiles deep
                # (24 x = 4 triples + 6 pairs)
                xa = 0
                for t8 in range(9):
                    if t8 % 3 < 2:  # triples via ps3
                        ps = ps3p.tile([128, 1536], F32, name="ps1", tag="ps3")
                        n = 3
                        for i in range(3):
                            x = xa + i
                            nc.tensor.matmul(
                                ps[:, 512 * i : 512 * i + 192],
                                tI[:, 128 * x : 128 * x + 128],
                                tB[0:42, :],
                                start=True, stop=True,
                            )
                        src = ps.rearrange("p (i n) -> p i n", i=3)[
                            :, :, 0:192
                        ].rearrange("p i (Q oyq) -> p i Q oyq", Q=4)
                    else:  # pairs via ps2
                        ps = ps2p.tile([128, 512], F32, name="ps1b", tag="ps2")
                        n = 2
                        for i in range(2):
                            x = xa + i
                            nc.tensor.matmul(
                                ps[:, 192 * i : 192 * i + 192],
                                tI[:, 128 * x : 128 * x + 128],
                                tB[0:42, :],
                                start=True, stop=True,
                            )
                        src = ps[:, 0:384].rearrange(
                            "p (i Q oyq) -> p i Q oyq", i=2, Q=4
                        )
                    copy(JAw[:, xa : xa + n], src)
                    xa += n
                for t4 in range(4):  # ch2, x-pair triples
                    ps = ps3p.tile([128, 1536], F32, name="ps1b", tag="ps3")
                    for i in range(3):
                        p = 3 * t4 + i
                        nc.tensor.matmul(
                            ps[:, 512 * i : 512 * i + 192],
                            tI[:, 3072 + 128 * p : 3072 + 128 * p + 128],
                            tB[0:42, :],
                            start=True, stop=True,
                        )
                    src = ps.rearrange("p (i n) -> p i n", i=3)[
                        :, :, 0:192
                    ].rearrange("p i (Q oyq) -> p i Q oyq", Q=4)
                    copy(JBv[0:42, 0, 3 * t4 : 3 * t4 + 3], src[0:42])
                    copy(JBv[0:42, 1, 3 * t4 : 3 * t4 + 3], src[64:106])

                # stage 2: one 8-oyq slab per (ch, wp); lhsT = J ch-band.
                def fill_s2(ch, wp):
                    if ch == 0:
                        Jt, p0 = J_A, 0
                    elif ch == 1:
                        Jt, p0 = J_A, 64
                    else:
                        Jt, p0 = J_B, 0
                    slab = slabs.tile([96, 1536], F16, name="slab")
                    slr = slab.rearrange("p (u j z) -> p u j z", u=4, j=2)
                    for u in range(4):
                        ps = ps2p.tile([128, 512], F32, name="ps2", tag="ps2")
                        for j in range(2):
                            q = 8 * wp + 2 * u + j
                            nc.tensor.matmul(
                                ps[0:96, 192 * j : 192 * j + 192],
                                Jt[p0 : p0 + 42, 96 * q : 96 * q + 96],
                                tB[p0 : p0 + 42, :],
                                start=True, stop=True,
                            )
                        copy(
                            slr[0:96, u],
                            ps[0:96, 0:384].rearrange("p (j z) -> p j z", j=2),
                        )
                    return slab

                # stage 3: expand x; 512-wide streams over the slab, psum
                # tile = 3 matmuls of one ox-block b, contiguous 1536 copy.
                # ob accumulates a wp-PAIR: free = (b 3, half 2, oo 8, z 192)
                # so each (Q, oxr, b) DMA row is one 6KB DRAM run.
                ob_box = [None]

                def do_s3(ch, wp, slab):
                    half = wp % 2
                    if half == 0:
                        ob_box[0] = obp.tile([128, 9216], F16, name="ob")
                    ob = ob_box[0]
                    for b in range(3):
                        ps = ps3p.tile([128, 1536], F32, name="ps3", tag="ps3")
                        for w in range(3):
                            nc.tensor.matmul(
                                ps[:, 512 * w : 512 * w + 512],
                                tBx[:, 128 * b : 128 * b + 128],
                                slab[0:96, 512 * w : 512 * w + 512],
                                start=True, stop=True,
                            )
                        copy(
                            ob[:, 3072 * b + 1536 * half : 3072 * b + 1536 * half + 1536],
                            ps[:],
                        )
                    if half == 1:
                        obd = ob.rearrange("p (b m) -> p b m", b=3)
                        for Q in range(4):
                            out_dma(
                                outv[ch, wp // 2, Q], obd[32 * Q : 32 * Q + 32]
                            )

                # software pipeline: issue s2 of group g+1 before s3 of g;
                # group g's out-DMAs fire after group g+1's copies are queued.
                prev = None
                for ch in range(3):
                    for wp in range(6):
                        slab = fill_s2(ch, wp)
                        flush_dmas()
                        if prev is not None:
                            do_s3(*prev)
                        prev = (ch, wp, slab)
                do_s3(*prev)
            flush_dmas()
    nc.compile()
    if reps == 1:
        _NC_CACHE = nc
    return nc


def make_inputs(v: np.ndarray) -> list[dict[str, np.ndarray]]:
    """Per-core input maps from the full (4, 3, 42, 42, 42) tensor."""
    B = _bspline_B()
    byT = np.zeros((128, 192), np.float16)
    byT[0:42] = B.T.astype(np.float16)
    byT[64:106] = B.T.astype(np.float16)
    ins = []
    for core in range(N_CORES):
        b, h = divmod(core, 2)
        c0 = 19 * h
        vs = v[b, :, c0 : c0 + 23, :, :].astype(np.float16)  # (ch, x, y, z)
        # blocks 0..23 (ch0+ch1): tI[y, 128x + 64ch + z]
        # blocks 24..35 (ch2):    tI[y, 3072 + 128p + 64e + z], x = 2p+e
        tia = np.zeros((42, 24, 2, 64), np.float16)
        tia[:, :23, 0, :42] = np.transpose(vs[0], (1, 0, 2))  # -> (y, x, z)
        tia[:, :23, 1, :42] = np.transpose(vs[1], (1, 0, 2))
        tib = np.zeros((42, 12, 2, 64), np.float16)
        tib.reshape(42, 24, 64)[:, :23, :42] = np.transpose(vs[2], (1, 0, 2))
        ti = np.concatenate(
            [tia.reshape(42, 3072), tib.reshape(42, 1536)], axis=1
        )
        Bxh = B[96 * h : 96 * h + 96, c0 : c0 + 23].astype(np.float16)  # (96ox, 23x)
        bx3 = np.zeros((4, 24, 3, 4, 32), np.float16)  # (Q, x, b, Q', oxr)
        for Q in range(4):
            for blk in range(3):
                bx3[Q, :23, blk, Q, :] = Bxh[32 * blk : 32 * blk + 32, :].T
        ins.append({
            "inp": np.ascontiguousarray(ti),
            "byT": byT,
            "bx3": np.ascontiguousarray(bx3.reshape(96, 384)),
        })
    return ins


def assemble(results: list[dict[str, np.ndarray]]) -> np.ndarray:
    full = np.empty((4, 3, 192, 192, 192), np.float32)
    for core in range(N_CORES):
        b, h = divmod(core, 2)
        full[b, :, 96 * h : 96 * h + 96, :, :] = results[core]["out"].astype(
            np.float32
        )
    return full


def kernel(v: np.ndarray) -> np.ndarray:
    v = np.ascontiguousarray(np.asarray(v, dtype=np.float32))
    assert v.shape == (4, 3, 42, 42, 42)
    nc = _build_nc()
    ins = make_inputs(v)
    res = run_bass_kernel_spmd(nc, ins, list(range(N_CORES)))
    return assemble(res.results)
